# revision 12
# baseline (speedup 1.0000x reference)
"""Single-head attention on 8 Trainium2 NeuronCores.

Problem: B=8, S=2048, WIDTH=1024, HEAD=64 single attention head.
Sharding: data-parallel over batch -- batch b runs on core b. No collectives.

Per-core pipeline (all matmuls in float32r -- full-rate fp32 on the PE):
  phase A: load x^T (host-pretransposed), project [Wk|Wq*scale] -> psum
           [K^T; Q^T] stacked [128, S]; Wv -> V^T [64, S].  Copy out with
           biases.  DMA-copy Q^T half down to partitions 0:64.  PE-transpose
           V^T into V chunks [128, 65] with a ones column appended.
  phase B: per q-block of 1024, per k-chunk of 128:
           scores^T = K^T_chunk.T @ Q^T  (PSUM [128, 1024])
           expS = exp(scores^T + mask_bias)   (ScalarE, from PSUM)
           O'^T += V'_chunk.T @ expS          (accumulate [65, 1024])
           row 64 of O'^T = softmax denominators (ones column of V').
           Transpose O'^T 128-col slices, normalize with per-partition
           reciprocal scale, DMA out rows.
"""

import os
from contextlib import ExitStack

import numpy as np

import concourse.bass as bass
import concourse.tile as tile
from concourse import mybir
from concourse.bass import ts

S = 2048
W = 1024
H = 64
N_CORES = 8
WC = W // 128  # 8 w-chunks
KC = S // 128  # 16 k-chunks
QB = 1024      # q-block size
NQB = S // QB  # 2
QT = QB // 128  # 8 q-tiles per block

F32 = mybir.dt.float32
F32R = mybir.dt.float32r
AF = mybir.ActivationFunctionType


def _emit(ctx, tc, xT, wkq, wv, bkq, bv, ident, mbias, vones, zeros64, out, rep=""):
    nc = tc.nc

    def pool(name, **kw):
        return ctx.enter_context(tc.tile_pool(name=name + rep, **kw))

    singles = pool("singles", bufs=1)
    wkq_sb = singles.tile([128, WC * 128], F32R)
    nc.sync.dma_start(out=wkq_sb, in_=wkq)
    wv_sb = singles.tile([128, WC * H], F32R)
    nc.sync.dma_start(out=wv_sb, in_=wv)
    bkq_sb = singles.tile([128, 1], F32)
    nc.sync.dma_start(out=bkq_sb, in_=bkq)
    bv_sb = singles.tile([64, 1], F32)
    nc.sync.dma_start(out=bv_sb, in_=bv)
    ident_sb = singles.tile([128, 128], F32R)
    nc.sync.dma_start(out=ident_sb, in_=ident)
    mbias_sb = singles.tile([128, KC], F32)
    nc.sync.dma_start(out=mbias_sb, in_=mbias)

    kq_sb = singles.tile([128, S], F32R)  # rows 0:64 = K^T, rows 64:128 = Q^T
    q_sb = singles.tile([64, S], F32R)    # Q^T copied down to partitions 0:64
    vT_sb = singles.tile([64, S], F32R)
    v_sb = singles.tile([128, KC, H + 1], F32R)  # V' chunks (+ones col)

    # ---------------- Phase A: projections ----------------
    with (
        tc.tile_pool(name="xp" + rep, bufs=3) as xp,
        tc.tile_pool(name="kqps" + rep, bufs=1, space="PSUM") as kqps_pool,
        tc.tile_pool(name="vtps" + rep, bufs=1, space="PSUM") as vtps_pool,
    ):
        kq_ps = kqps_pool.tile([128, S], F32)  # 4 banks
        vT_ps = vtps_pool.tile([64, S], F32)   # 4 banks
        for c in range(WC):
            xt = xp.tile([128, S], F32R)
            nc.sync.dma_start(out=xt, in_=xT[ts(c, 128), :])
            for j in range(S // 512):
                nc.tensor.matmul(
                    kq_ps[:, ts(j, 512)],
                    wkq_sb[:, ts(c, 128)],
                    xt[:, ts(j, 512)],
                    start=(c == 0),
                    stop=(c == WC - 1),
                )
            for j in range(S // 512):
                nc.tensor.matmul(
                    vT_ps[:, ts(j, 512)],
                    wv_sb[:, ts(c, H)],
                    xt[:, ts(j, 512)],
                    start=(c == 0),
                    stop=(c == WC - 1),
                )
        nc.vector.tensor_scalar_add(kq_sb, kq_ps, bkq_sb)
        nc.scalar.activation(vT_sb, vT_ps, AF.Identity, bias=bv_sb, scale=1.0)

    # Q^T to partitions 0:64 (partition remap -> DMA sbuf->sbuf)
    nc.sync.dma_start(out=q_sb, in_=kq_sb[64:128, :])

    # V^T -> V natural chunks
    with tc.tile_pool(name="vtr" + rep, bufs=2, space="PSUM") as vtr:
        for t in range(KC):
            vp = vtr.tile([128, H], F32R)
            nc.tensor.transpose(vp, vT_sb[:, ts(t, 128)], ident_sb[0:64, 0:64])
            nc.vector.tensor_copy(v_sb[:, t, 0:H], vp)
    nc.sync.dma_start(
        out=v_sb[:, :, H : H + 1],
        in_=vones.rearrange("p (k one) -> p k one", one=1),
    )

    # ---------------- Phase B: attention ----------------
    sc_pool = pool("scps", bufs=3, space="PSUM")
    ot_pool = pool("otps", bufs=1, space="PSUM")
    es_pool = pool("es", bufs=4)
    osb_pool = pool("osb", bufs=2)
    rec_pool = pool("rec", bufs=4)
    out_pool = pool("outsb", bufs=2)

    out_v = out.rearrange("(qb t p) h -> qb p t h", p=128, t=QT)

    for qb in range(NQB):
        q0 = qb * QB
        oT_ps = ot_pool.tile([H + 1, QB], F32)  # 2 banks
        for k in range(KC):
            sc = sc_pool.tile([128, QB], F32, tag="scps" + rep)  # 2 banks
            for h in range(QB // 512):
                nc.tensor.matmul(
                    sc[:, ts(h, 512)],
                    kq_sb[0:64, ts(k, 128)],
                    q_sb[:, q0 + h * 512 : q0 + (h + 1) * 512],
                )
            es = es_pool.tile([128, QB], F32R)
            nc.scalar.activation(
                es, sc, AF.Exp, bias=mbias_sb[:, k : k + 1], scale=1.0
            )
            for h in range(QB // 512):
                nc.tensor.matmul(
                    oT_ps[:, ts(h, 512)],
                    v_sb[:, k, :],
                    es[:, ts(h, 512)],
                    start=(k == 0),
                    stop=(k == KC - 1),
                )
        # transpose-mode matmul needs K in {32,64,128}: pad O'^T to 128
        # partitions (rows 65:127 zeroed; they land in unread out columns)
        oT_s = osb_pool.tile([128, QB], F32R)
        nc.sync.dma_start(out=oT_s[H : 128, :], in_=zeros64)
        nc.vector.tensor_copy(oT_s[0 : H + 1, :], oT_ps)
        ob = out_pool.tile([128, QT, H], F32)
        for t in range(QT):
            op = sc_pool.tile([128, 128], F32R, tag="scps" + rep)
            nc.tensor.transpose(op, oT_s[:, ts(t, 128)], ident_sb)
            rec = rec_pool.tile([128, 1], F32)
            nc.vector.reciprocal(rec, op[:, H : H + 1])
            nc.scalar.activation(ob[:, t, :], op[:, 0:H], AF.Copy, scale=rec)
        nc.sync.dma_start(out=out_v[qb], in_=ob)


def split_multi_waits(nc):
    """This walrus build encodes at most ONE sync-wait per hw instruction.
    Hoist all but the last wait of any multi-wait instruction into standalone
    single-wait NoOps on the same engine queue (semantically identical:
    engine-queue execution is in-order)."""
    import bass_rust

    ctr = 0
    for blk in nc.m.functions[0].blocks:
        insts = blk.instructions
        out = []
        changed = False
        for inst in insts:
            si = inst.sync_info
            if si is not None and si.on_wait and len(si.on_wait) > 1:
                waits = list(si.on_wait)
                for w in waits[:-1]:
                    ctr += 1
                    nop = mybir.InstNoOp(name=f"WSPLIT-{ctr}", ins=[], outs=[])
                    nop.engine = inst.engine
                    nop.sync_info = bass_rust.SyncInfo(on_wait=[w], on_update=[])
                    out.append(nop)
                inst.sync_info = bass_rust.SyncInfo(
                    on_wait=[waits[-1]], on_update=list(si.on_update or [])
                )
                out.append(inst)
                changed = True
            else:
                out.append(inst)
        if changed:
            insts[:] = out
    return nc


def build_bass(split=True, repeat=1):
    nc = bass.Bass("TRN2", target_bir_lowering=False, debug=False)
    xT = nc.dram_tensor("xT", [W, S], F32R, kind="ExternalInput").ap()
    wkq = nc.dram_tensor("wkq", [128, WC * 128], F32R, kind="ExternalInput").ap()
    wv = nc.dram_tensor("wv", [128, WC * H], F32R, kind="ExternalInput").ap()
    bkq = nc.dram_tensor("bkq", [128, 1], F32, kind="ExternalInput").ap()
    bv = nc.dram_tensor("bv", [64, 1], F32, kind="ExternalInput").ap()
    ident = nc.dram_tensor("ident", [128, 128], F32R, kind="ExternalInput").ap()
    mbias = nc.dram_tensor("mbias", [128, KC], F32, kind="ExternalInput").ap()
    vones = nc.dram_tensor("vones", [128, KC], F32R, kind="ExternalInput").ap()
    zeros64 = nc.dram_tensor("zeros64", [H, QB], F32R, kind="ExternalInput").ap()
    out = nc.dram_tensor("out", [S, H], F32, kind="ExternalOutput").ap()
    with tile.TileContext(nc) as tc:
        for r in range(repeat):
            with ExitStack() as ctx:
                _emit(
                    ctx, tc, xT, wkq, wv, bkq, bv, ident, mbias, vones,
                    zeros64, out, rep=(f"_r{r}" if r else ""),
                )
    if split:
        split_multi_waits(nc)
    return nc


def prep_in_maps(x, attn_mask, Wq, bq, Wk, bk, Wv, bv):
    x = np.asarray(x, dtype=np.float32)
    attn_mask = np.asarray(attn_mask)
    Wq = np.asarray(Wq, dtype=np.float32)
    Wk = np.asarray(Wk, dtype=np.float32)
    Wv = np.asarray(Wv, dtype=np.float32)
    bq = np.asarray(bq, dtype=np.float32)
    bk = np.asarray(bk, dtype=np.float32)
    bv = np.asarray(bv, dtype=np.float32)

    scale = np.float32(H) ** np.float32(-0.5)
    # [Wk | Wq*scale] -> per-w-chunk stationary layout [128, WC*128]
    wkq = np.concatenate([Wk, Wq * scale], axis=1)  # [W, 128]
    wkq = np.ascontiguousarray(
        wkq.reshape(WC, 128, 128).transpose(1, 0, 2).reshape(128, WC * 128)
    )
    wv_h = np.ascontiguousarray(
        Wv.reshape(WC, 128, H).transpose(1, 0, 2).reshape(128, WC * H)
    )
    bkq = np.concatenate([bk, bq * scale]).reshape(128, 1)
    bv_h = bv.reshape(H, 1)
    ident = np.eye(128, dtype=np.float32)

    in_maps = []
    for c in range(N_CORES):
        xT_c = np.ascontiguousarray(x[c].T)  # [W, S]
        m = attn_mask[c].astype(np.float32)  # [S]
        mb = np.where(m != 0, np.float32(0.0), np.float32(-1e30))
        mbias = np.ascontiguousarray(mb.reshape(KC, 128).T)  # [128, KC]
        in_maps.append(
            {
                "xT": xT_c,
                "wkq": wkq,
                "wv": wv_h,
                "bkq": np.ascontiguousarray(bkq),
                "bv": np.ascontiguousarray(bv_h),
                "ident": ident,
                "mbias": mbias,
                "vones": np.ones((128, KC), dtype=np.float32),
                "zeros64": np.zeros((H, QB), dtype=np.float32),
            }
        )
    return in_maps


def run(x, attn_mask, Wq, bq, Wk, bk, Wv, bv, trace=False, **rb_kwargs):
    from concourse.bass_utils import run_bass_kernel_spmd

    nc = build_bass()
    in_maps = prep_in_maps(x, attn_mask, Wq, bq, Wk, bk, Wv, bv)
    res = run_bass_kernel_spmd(
        nc, in_maps, core_ids=list(range(N_CORES)), trace=trace, **rb_kwargs
    )
    out = np.stack([r["out"] for r in res.results]).astype(np.float32)
    return out, res


def kernel(x, attn_mask, Wq, bq, Wk, bk, Wv, bv):
    out, _ = run(x, attn_mask, Wq, bq, Wk, bk, Wv, bv, trace=False)
    return out


# revision 15
# speedup vs baseline: 1.5520x; 1.5520x over previous
"""Single-head attention on 8 Trainium2 NeuronCores.

Problem: B=8, S=2048, WIDTH=1024, HEAD=64 single attention head.
Sharding: data-parallel over batch -- batch b runs on core b. No collectives.

Per-core pipeline (all matmuls in float32r -- full-rate fp32 on the PE):
  phase A: load x^T (host-pretransposed), project [Wk|Wq*scale] -> psum
           [K^T; Q^T] stacked [128, S]; Wv -> V^T [64, S].  Copy out with
           biases.  DMA-copy Q^T half down to partitions 0:64.  PE-transpose
           V^T into V chunks [128, 65] with a ones column appended.
  phase B: per q-block of 1024, per k-chunk of 128:
           scores^T = K^T_chunk.T @ Q^T  (PSUM [128, 1024])
           expS = exp(scores^T + mask_bias)   (ScalarE, from PSUM)
           O'^T += V'_chunk.T @ expS          (accumulate [65, 1024])
           row 64 of O'^T = softmax denominators (ones column of V').
           Transpose O'^T 128-col slices, normalize with per-partition
           reciprocal scale, DMA out rows.
"""

import os
from contextlib import ExitStack

import numpy as np

import concourse.bass as bass
import concourse.tile as tile
from concourse import mybir
from concourse.bass import ts

S = 2048
W = 1024
H = 64
N_CORES = 8
WC = W // 128  # 8 w-chunks
KC = S // 128  # 16 k-chunks
QB = 1024      # q-block size
NQB = S // QB  # 2
QT = QB // 128  # 8 q-tiles per block

F32 = mybir.dt.float32
F32R = mybir.dt.float32r
AF = mybir.ActivationFunctionType


def _emit(ctx, tc, xT, wkq, wv, bkq, bv, ident, mbias, vones, zeros64, out,
          rep="", probe=None):
    nc = tc.nc

    def pool(name, **kw):
        return ctx.enter_context(tc.tile_pool(name=name + rep, **kw))

    singles = pool("singles", bufs=1)
    wkq_sb = singles.tile([128, WC * 128], F32R)
    nc.sync.dma_start(out=wkq_sb, in_=wkq)
    wv_sb = singles.tile([128, WC * H], F32R)
    nc.sync.dma_start(out=wv_sb, in_=wv)
    bkq_sb = singles.tile([128, 1], F32)
    nc.sync.dma_start(out=bkq_sb, in_=bkq)
    bv_sb = singles.tile([64, 1], F32)
    nc.sync.dma_start(out=bv_sb, in_=bv)
    ident_sb = singles.tile([128, 128], F32R)
    nc.sync.dma_start(out=ident_sb, in_=ident)
    mbias_sb = singles.tile([128, KC], F32)
    nc.sync.dma_start(out=mbias_sb, in_=mbias)

    kq_sb = singles.tile([128, S], F32R)  # rows 0:64 = K^T, rows 64:128 = Q^T
    q_sb = singles.tile([64, S], F32R)    # Q^T copied down to partitions 0:64
    vT_sb = singles.tile([64, S], F32R)
    v_sb = singles.tile([128, KC, H + 1], F32R)  # V' chunks (+ones col)

    # ---------------- Phase A: projections ----------------
    with (
        tc.tile_pool(name="xp" + rep, bufs=3) as xp,
        tc.tile_pool(name="kqps" + rep, bufs=1, space="PSUM") as kqps_pool,
        tc.tile_pool(name="vtps" + rep, bufs=1, space="PSUM") as vtps_pool,
    ):
        kq_ps = kqps_pool.tile([128, S], F32)  # 4 banks
        vT_ps = vtps_pool.tile([64, S], F32)   # 4 banks
        for c in range(WC):
            xt = xp.tile([128, S], F32R)
            nc.sync.dma_start(out=xt, in_=xT[ts(c, 128), :])
            for j in range(S // 512):
                nc.tensor.matmul(
                    kq_ps[:, ts(j, 512)],
                    wkq_sb[:, ts(c, 128)],
                    xt[:, ts(j, 512)],
                    start=(c == 0),
                    stop=(c == WC - 1),
                )
            for j in range(S // 512):
                nc.tensor.matmul(
                    vT_ps[:, ts(j, 512)],
                    wv_sb[:, ts(c, H)],
                    xt[:, ts(j, 512)],
                    start=(c == 0),
                    stop=(c == WC - 1),
                )
        nc.vector.tensor_scalar_add(kq_sb, kq_ps, bkq_sb)
        nc.scalar.activation(vT_sb, vT_ps, AF.Identity, bias=bv_sb, scale=1.0)

    # Q^T to partitions 0:64 (partition remap -> DMA sbuf->sbuf)
    nc.sync.dma_start(out=q_sb, in_=kq_sb[64:128, :])

    # V^T -> V natural chunks
    with tc.tile_pool(name="vtr" + rep, bufs=2, space="PSUM") as vtr:
        for t in range(KC):
            vp = vtr.tile([128, H], F32R)
            nc.tensor.transpose(vp, vT_sb[:, ts(t, 128)], ident_sb[0:64, 0:64])
            nc.vector.tensor_copy(v_sb[:, t, 0:H], vp)
    nc.sync.dma_start(
        out=v_sb[:, :, H : H + 1],
        in_=vones.rearrange("p (k one) -> p k one", one=1),
    )

    if probe == "A":
        # timing probe: stop after phase A; dump q_sb bytes as the output
        nc.sync.dma_start(
            out=out.rearrange("(a p) h -> p a h", p=128),
            in_=v_sb[:, :, 0:H].bitcast(F32),
        )
        return

    # ---------------- Phase B: attention ----------------
    sc_pool = pool("scps", bufs=3, space="PSUM")
    ot_pool = pool("otps", bufs=1, space="PSUM")
    es_pool = pool("es", bufs=4)
    osb_pool = pool("osb", bufs=2)
    rec_pool = pool("rec", bufs=4)
    out_pool = pool("outsb", bufs=2)

    out_v = out.rearrange("(qb t p) h -> qb p t h", p=128, t=QT)

    for qb in range(NQB):
        q0 = qb * QB
        oT_ps = ot_pool.tile([H + 1, QB], F32)  # 2 banks
        for k in range(KC):
            sc = sc_pool.tile([128, QB], F32, tag="scps" + rep)  # 2 banks
            for h in range(QB // 512):
                nc.tensor.matmul(
                    sc[:, ts(h, 512)],
                    kq_sb[0:64, ts(k, 128)],
                    q_sb[:, q0 + h * 512 : q0 + (h + 1) * 512],
                )
            es = es_pool.tile([128, QB], F32R)
            nc.scalar.activation(
                es, sc, AF.Exp, bias=mbias_sb[:, k : k + 1], scale=1.0
            )
            if probe == "AB":
                continue
            for h in range(QB // 512):
                nc.tensor.matmul(
                    oT_ps[:, ts(h, 512)],
                    v_sb[:, k, :],
                    es[:, ts(h, 512)],
                    start=(k == 0),
                    stop=(k == KC - 1),
                )
        if probe == "AB":
            if qb == NQB - 1:
                nc.sync.dma_start(
                    out=out.rearrange("(a p) h -> p a h", p=128),
                    in_=es.rearrange("p (a h) -> p a h", h=64)[:, 0:KC, :].bitcast(F32),
                )
            continue
        # transpose-mode matmul needs K in {32,64,128}: pad O'^T to 128
        # partitions (rows 65:127 zeroed; they land in unread out columns)
        oT_s = osb_pool.tile([128, QB], F32R)
        nc.sync.dma_start(out=oT_s[H : 128, :], in_=zeros64)
        nc.vector.tensor_copy(oT_s[0 : H + 1, :], oT_ps)
        ob = out_pool.tile([128, QT, H], F32)
        for t in range(QT):
            op = sc_pool.tile([128, 128], F32R, tag="scps" + rep)
            nc.tensor.transpose(op, oT_s[:, ts(t, 128)], ident_sb)
            rec = rec_pool.tile([128, 1], F32)
            nc.vector.reciprocal(rec, op[:, H : H + 1])
            nc.scalar.activation(ob[:, t, :], op[:, 0:H], AF.Copy, scale=rec)
        nc.sync.dma_start(out=out_v[qb], in_=ob)


def split_multi_waits(nc):
    """This walrus build encodes at most ONE sync-wait per hw instruction.
    Hoist all but the last wait of any multi-wait instruction into standalone
    single-wait NoOps on the same engine queue (semantically identical:
    engine-queue execution is in-order)."""
    import bass_rust

    ctr = 0
    for blk in nc.m.functions[0].blocks:
        insts = blk.instructions
        out = []
        changed = False
        for inst in insts:
            si = inst.sync_info
            if si is not None and si.on_wait and len(si.on_wait) > 1:
                waits = list(si.on_wait)
                for w in waits[:-1]:
                    ctr += 1
                    nop = mybir.InstNoOp(name=f"WSPLIT-{ctr}", ins=[], outs=[])
                    nop.engine = inst.engine
                    nop.sync_info = bass_rust.SyncInfo(on_wait=[w], on_update=[])
                    out.append(nop)
                inst.sync_info = bass_rust.SyncInfo(
                    on_wait=[waits[-1]], on_update=list(si.on_update or [])
                )
                out.append(inst)
                changed = True
            else:
                out.append(inst)
        if changed:
            insts[:] = out
    return nc


def build_bass(split=True, repeat=1, probe=None):
    nc = bass.Bass("TRN2", target_bir_lowering=False, debug=False)
    xT = nc.dram_tensor("xT", [W, S], F32R, kind="ExternalInput").ap()
    wkq = nc.dram_tensor("wkq", [128, WC * 128], F32R, kind="ExternalInput").ap()
    wv = nc.dram_tensor("wv", [128, WC * H], F32R, kind="ExternalInput").ap()
    bkq = nc.dram_tensor("bkq", [128, 1], F32, kind="ExternalInput").ap()
    bv = nc.dram_tensor("bv", [64, 1], F32, kind="ExternalInput").ap()
    ident = nc.dram_tensor("ident", [128, 128], F32R, kind="ExternalInput").ap()
    mbias = nc.dram_tensor("mbias", [128, KC], F32, kind="ExternalInput").ap()
    vones = nc.dram_tensor("vones", [128, KC], F32R, kind="ExternalInput").ap()
    zeros64 = nc.dram_tensor("zeros64", [H, QB], F32R, kind="ExternalInput").ap()
    out = nc.dram_tensor("out", [S, H], F32, kind="ExternalOutput").ap()
    with tile.TileContext(nc) as tc:
        for r in range(repeat):
            with ExitStack() as ctx:
                _emit(
                    ctx, tc, xT, wkq, wv, bkq, bv, ident, mbias, vones,
                    zeros64, out, rep=(f"_r{r}" if r else ""), probe=probe,
                )
    if split:
        split_multi_waits(nc)
    return nc


def prep_in_maps(x, attn_mask, Wq, bq, Wk, bk, Wv, bv):
    x = np.asarray(x, dtype=np.float32)
    attn_mask = np.asarray(attn_mask)
    Wq = np.asarray(Wq, dtype=np.float32)
    Wk = np.asarray(Wk, dtype=np.float32)
    Wv = np.asarray(Wv, dtype=np.float32)
    bq = np.asarray(bq, dtype=np.float32)
    bk = np.asarray(bk, dtype=np.float32)
    bv = np.asarray(bv, dtype=np.float32)

    scale = np.float32(H) ** np.float32(-0.5)
    # [Wk | Wq*scale] -> per-w-chunk stationary layout [128, WC*128]
    wkq = np.concatenate([Wk, Wq * scale], axis=1)  # [W, 128]
    wkq = np.ascontiguousarray(
        wkq.reshape(WC, 128, 128).transpose(1, 0, 2).reshape(128, WC * 128)
    )
    wv_h = np.ascontiguousarray(
        Wv.reshape(WC, 128, H).transpose(1, 0, 2).reshape(128, WC * H)
    )
    bkq = np.concatenate([bk, bq * scale]).reshape(128, 1)
    bv_h = bv.reshape(H, 1)
    ident = np.eye(128, dtype=np.float32)

    in_maps = []
    for c in range(N_CORES):
        xT_c = np.ascontiguousarray(x[c].T)  # [W, S]
        m = attn_mask[c].astype(np.float32)  # [S]
        mb = np.where(m != 0, np.float32(0.0), np.float32(-1e30))
        mbias = np.ascontiguousarray(mb.reshape(KC, 128).T)  # [128, KC]
        in_maps.append(
            {
                "xT": xT_c,
                "wkq": wkq,
                "wv": wv_h,
                "bkq": np.ascontiguousarray(bkq),
                "bv": np.ascontiguousarray(bv_h),
                "ident": ident,
                "mbias": mbias,
                "vones": np.ones((128, KC), dtype=np.float32),
                "zeros64": np.zeros((H, QB), dtype=np.float32),
            }
        )
    return in_maps


def run(x, attn_mask, Wq, bq, Wk, bk, Wv, bv, trace=False, **rb_kwargs):
    from concourse.bass_utils import run_bass_kernel_spmd

    nc = build_bass()
    in_maps = prep_in_maps(x, attn_mask, Wq, bq, Wk, bk, Wv, bv)
    res = run_bass_kernel_spmd(
        nc, in_maps, core_ids=list(range(N_CORES)), trace=trace, **rb_kwargs
    )
    out = np.stack([r["out"] for r in res.results]).astype(np.float32)
    return out, res


def kernel(x, attn_mask, Wq, bq, Wk, bk, Wv, bv):
    out, _ = run(x, attn_mask, Wq, bq, Wk, bk, Wv, bv, trace=False)
    return out
